# revision 20
# baseline (speedup 1.0000x reference)
"""Multi-head attention (B=8, N=1024, H=12, D=64, C=768) on 8 trn2 cores.

Sharding: data-parallel over batch. Core b computes attention for x[b];
weights are replicated. No collectives.

Per-core dataflow (all matmul operands float32r = full PE rate, fp32 bits):
  phase 1a: qkT[1536 x N] = W_qkv[:, :1536].T @ x^T    (d-major Q^T, K^T)
  phase 1b: v[N x 768]    = x @ W_qkv[:, 1536:]        (+ ones column per head)
  phase 2 (per head pair, heads 2t/2t+1 packed at partitions 0:64/64:128):
     S^T[m,n] = k^T.T @ q^T            (K=64 row-group packed pairs)
     P^T = exp(S^T / 8)                 (ScalarE, one [128,1024] op per m)
     outT[65,n] += v_aug[m].T @ P^T     (row 64 = rowsum via ones column)
     hT = outT[0:64] * bcast(1/rowsum)  (DVE mult; hT aliases the dead Q tile)
  phase 3: y = hT.T @ W_proj
"""
from contextlib import nullcontext

import numpy as np

import concourse.bass as bass
import concourse.mybir as mybir
import concourse.tile as tile
from concourse import bacc
from concourse.bass_utils import run_bass_kernel_spmd

F32R = mybir.dt.float32r
F32 = mybir.dt.float32

B, N, C = 8, 1024, 768
H, D = 12, 64
HID = H * D  # 768
KT = C // 128          # 6 feature k-tiles
MT = N // 128          # 8 sequence m-tiles
SCALE = D ** -0.5      # 0.125

_cached_nc = None

DEFAULT_OPTS = dict(
    s_bufs=2, acc_bufs=2, mm1_bufs=2, pt_bufs=4,
    eager_acc_evict=True, interleave_loads=True, proj_dual_pool=False,
    hoist_pair0=True,
)


def build_program(repeats=1, phases=("qk", "v", "attn", "proj"), **opts):
    o = dict(DEFAULT_OPTS, **opts)
    nc = bacc.Bacc(None, target_bir_lowering=False)

    xT_d = nc.dram_tensor("xT", [C, N], F32R, kind="ExternalInput")
    wqkv_d = nc.dram_tensor("wqkv", [C, 3 * HID], F32R, kind="ExternalInput")
    wproj_d = nc.dram_tensor("wproj", [HID, C], F32R, kind="ExternalInput")
    y_d = nc.dram_tensor("y", [N, C], F32, kind="ExternalOutput")

    with tile.TileContext(nc) as tc:
        with tc.tile_pool(name="persist", bufs=1) as persist, \
             tc.tile_pool(name="pt_pool", bufs=o["pt_bufs"]) as pt_pool, \
             tc.tile_pool(name="nrm_pool", bufs=3) as nrm_pool, \
             tc.tile_pool(name="y_pool", bufs=2) as y_pool, \
             tc.tile_pool(name="ps_a", bufs=o["mm1_bufs"], space="PSUM") as ps_a, \
             tc.tile_pool(name="ps_s", bufs=o["s_bufs"], space="PSUM") as ps_s, \
             tc.tile_pool(name="ps_acc", bufs=o["acc_bufs"], space="PSUM") as ps_acc:

            # ---- resident loads (emission order = DMA priority) ----
            xt = [persist.tile([128, N], F32R, name=f"xt{k}", tag=f"xt{k}")
                  for k in range(KT)]
            wqk = [persist.tile([128, 2 * HID], F32R, name=f"wqk{k}", tag=f"wqk{k}")
                   for k in range(KT)]
            wv = [persist.tile([128, HID], F32R, name=f"wv{k}", tag=f"wv{k}")
                  for k in range(KT)]
            if o["interleave_loads"]:
                for k in range(KT):
                    nc.sync.dma_start(xt[k][:], xT_d[k * 128:(k + 1) * 128, :])
                    nc.sync.dma_start(wqk[k][:],
                                      wqkv_d[k * 128:(k + 1) * 128, :2 * HID])
                for k in range(KT):
                    nc.sync.dma_start(wv[k][:], wqkv_d[k * 128:(k + 1) * 128, 2 * HID:])
            else:
                for k in range(KT):
                    nc.sync.dma_start(xt[k][:], xT_d[k * 128:(k + 1) * 128, :])
                for k in range(KT):
                    nc.sync.dma_start(wqk[k][:],
                                      wqkv_d[k * 128:(k + 1) * 128, :2 * HID])
                for k in range(KT):
                    nc.sync.dma_start(wv[k][:], wqkv_d[k * 128:(k + 1) * 128, 2 * HID:])

            # warm the exp table set during the DMA prefix (the ACT
            # table load otherwise lands on the first real exp)
            warm = persist.tile([1, 8], F32, name="warm", tag="warm")
            nc.gpsimd.memset(warm[:], 0.0)
            nc.scalar.activation(warm[:], warm[:],
                                 mybir.ActivationFunctionType.Exp)

            qkT = [persist.tile([128, N], F32R, name=f"qkT{t}", tag=f"qkT{t}")
                   for t in range(12)]
            v_aug = [persist.tile([128, H, D + 1], F32R, name=f"vaug{m}", tag=f"vaug{m}")
                     for m in range(MT)]
            hT = qkT[:6]  # normalized outputs overwrite the dead Q tiles

            # ---- phase 1a: one qkT tile (output rows = qkv cols t*128..) ----
            def qk_tile(t):
                for nh in range(2):
                    ps = ps_a.tile([128, 512], F32, name="ps_qk", tag="mm1")
                    for k in range(KT):
                        nc.tensor.matmul(ps[:], wqk[k][:, t * 128:(t + 1) * 128],
                                         xt[k][:, nh * 512:(nh + 1) * 512],
                                         start=(k == 0), stop=(k == KT - 1))
                    nc.vector.tensor_copy(qkT[t][:, nh * 512:(nh + 1) * 512], ps[:])

            # ---- phase 1b: v tiles ----
            def v_tile(m):
                for vh in range(2):
                    ps = ps_a.tile([128, 384], F32, name="ps_v", tag="mm1")
                    for k in range(KT):
                        nc.tensor.matmul(ps[:], xt[k][:, m * 128:(m + 1) * 128],
                                         wv[k][:, vh * 384:(vh + 1) * 384],
                                         start=(k == 0), stop=(k == KT - 1))
                    dst = v_aug[m][:, vh * 6:(vh + 1) * 6, 0:D]
                    nc.vector.tensor_copy(dst, ps[:].rearrange("p (h d) -> p h d", d=D))
                nc.gpsimd.memset(v_aug[m][:, :, D:D + 1].bitcast(F32), 1.0)

            # ---- phase 2: attention for head pair (2t, 2t+1) ----
            def attention(t, hoist=False):
                qT_t, kT_t = qkT[t], qkT[6 + t]
                for nh in range(2):
                    nsl = slice(nh * 512, (nh + 1) * 512)
                    acc = [ps_acc.tile([D + 1, 512], F32, name="acc", tag="acc")
                           for _ in range(2)]
                    for m in range(MT):
                        msl = slice(m * 128, (m + 1) * 128)
                        # both heads' S^T m-tile in one 2-bank psum tile;
                        # one [128,1024] exp serves both.
                        with tc.high_priority() if hoist else nullcontext():
                            s_ps = ps_s.tile([128, 1024], F32, name="s_ps", tag="s")
                            for j in range(2):
                                psl = slice(j * 64, (j + 1) * 64)
                                nc.tensor.matmul(s_ps[:, j * 512:(j + 1) * 512],
                                                 kT_t[psl, msl], qT_t[psl, nsl],
                                                 start=True, stop=True)
                            p_sb = pt_pool.tile([128, 1024], F32R, name="p_sb", tag="p")
                            nc.scalar.activation(p_sb[:], s_ps[:],
                                                 mybir.ActivationFunctionType.Exp,
                                                 scale=SCALE)
                        for j in range(2):
                            nc.tensor.matmul(acc[j][:], v_aug[m][:, 2 * t + j, :],
                                             p_sb[:, j * 512:(j + 1) * 512],
                                             start=(m == 0), stop=(m == MT - 1))
                    # normalize: rowsum sits in acc[j] row 64. HW
                    # partition_broadcast reads physical partition 0, so each
                    # reciprocal lives in its own [1, 512] tile.
                    for j in range(2):
                        rs = nrm_pool.tile([1, 512], F32, name="rs", tag="rs")
                        nc.vector.reciprocal(rs[0:1, :], acc[j][D:D + 1, :])
                        bc = nrm_pool.tile([64, 512], F32, name="bc", tag="bc")
                        nc.gpsimd.partition_broadcast(bc[:], rs[0:1, :])
                        if o["eager_acc_evict"]:
                            ev = pt_pool.tile([64, 512], F32, name="ev", tag="ev")
                            nc.vector.tensor_copy(ev[:], acc[j][0:D, :])
                            nc.vector.tensor_mul(hT[t][j * 64:(j + 1) * 64, nsl],
                                                 ev[:], bc[:])
                        else:
                            nc.vector.tensor_mul(hT[t][j * 64:(j + 1) * 64, nsl],
                                                 acc[j][0:D, :], bc[:])

            # ---- phase 3: y = hT.T @ W_proj ----
            def proj(m):
                for ph in range(2):
                    if o["proj_dual_pool"] and ph == 1:
                        ps = ps_s.tile([128, 384], F32, name="ps_y2", tag="s")
                    else:
                        ps = ps_a.tile([128, 384], F32, name="ps_y", tag="mm1")
                    for k in range(KT):
                        nc.tensor.matmul(ps[:], hT[k][:, m * 128:(m + 1) * 128],
                                         wp[k][:, ph * 384:(ph + 1) * 384],
                                         start=(k == 0), stop=(k == KT - 1))
                    y_sb = y_pool.tile([128, 384], F32, name="y_sb", tag="y")
                    if o.get("y_evict_dve"):
                        nc.vector.tensor_copy(y_sb[:], ps[:])
                    else:
                        nc.scalar.copy(y_sb[:], ps[:])
                    nc.sync.dma_start(
                        y_d[m * 128:(m + 1) * 128, ph * 384:(ph + 1) * 384], y_sb[:])

            for _ in range(repeats):
                if "qk" in phases:
                    qk_tile(0)
                    qk_tile(6)
                if "v" in phases:
                    for m in range(MT):
                        v_tile(m)
                if "qk" in phases and "attn" in phases:
                    # pair 0's S^T/exp get hoisted over the v-phase PE work
                    attention(0, hoist=o["hoist_pair0"])
                    for t in range(1, 6):
                        qk_tile(t)
                        qk_tile(6 + t)
                        attention(t)
                elif "qk" in phases:
                    for t in range(1, 6):
                        qk_tile(t)
                        qk_tile(6 + t)
                if "proj" in phases:
                    wp = [persist.tile([128, C], F32R, name=f"wp{k}", tag=f"wp{k}")
                          for k in range(KT)]
                    for k in range(KT):
                        nc.sync.dma_start(wp[k][:], wproj_d[k * 128:(k + 1) * 128, :])
                    for m in range(MT):
                        proj(m)

    nc.compile()
    return nc


def _run(inputs, trace=False, trace_kwargs=None):
    global _cached_nc
    x = np.asarray(inputs["x"], dtype=np.float32)
    wqkv = np.ascontiguousarray(np.asarray(inputs["W_qkv"], dtype=np.float32))
    wproj = np.ascontiguousarray(np.asarray(inputs["W_proj"], dtype=np.float32))
    xT = np.ascontiguousarray(x.transpose(0, 2, 1))  # [B, C, N]

    if _cached_nc is None:
        _cached_nc = build_program()
    nc = _cached_nc

    in_maps = [{"xT": xT[b], "wqkv": wqkv, "wproj": wproj} for b in range(B)]
    kwargs = {}
    if trace:
        kwargs["trace"] = True
        if trace_kwargs:
            kwargs.update(trace_kwargs)
    try:
        res = run_bass_kernel_spmd(nc, in_maps, core_ids=list(range(B)), **kwargs)
    except Exception:
        # transient axon/PJRT hiccups happen; one retry
        res = run_bass_kernel_spmd(nc, in_maps, core_ids=list(range(B)), **kwargs)
    out = np.stack([r["y"] for r in res.results], axis=0)
    return out, res


def kernel(**inputs):
    out, _ = _run(inputs)
    return out
